# revision 1
# baseline (speedup 1.0000x reference)
"""Trainium2 Bass kernel for nn_BiologicalNormalization.

Math: three chained per-sample LayerNorms (affine params gathered per-sample
by id on the host). The trailing gated blend ``x*sigmoid(xW+b) +
x*(1-sigmoid(xW+b))`` is mathematically the identity, so the kernel returns
the triple-LayerNorm result directly.

Distribution: pure data parallelism - batch 2048 is split into 8 shards of
256 samples, one per NeuronCore. Per-id affine tables are gathered to
per-sample rows on the host (tiny), so each core only sees dense tensors.

Per-core schedule (partition dim = 128 samples, free dim = D=512, sequence
positions in chunks of K=8). Measured on this target, per-instruction
overhead (~1-2 us) dominates over engine throughput, so the kernel minimizes
instruction count: every elementwise op is K-fused across the whole chunk,
statistics use one K-fused reduce + one K-fused square + reduce, centering
uses free-dim-broadcast multiplies (z = y*r - m*r with 0-stride APs) instead
of per-slice tensor_scalar ops, and the mean/rstd finalization works in raw
sums (V = D*Sum(y^2) - Sum(y)^2) to save ops. A 5-stage software pipeline
(load / x-stats / LN1 / LN2 / LN3+store) keeps the in-order engines from
head-of-line blocking. Intermediates are bf16 (rel-err budget 2e-2; measured
~6e-3); statistics accumulate in f32.
"""

import contextlib

import ml_dtypes
import numpy as np

import concourse.bass as bass
import concourse.bacc as bacc
import concourse.mybir as mybir
from concourse.tile import TileContext

NCORES = 8
B, S, D = 2048, 128, 512
BS = B // NCORES  # samples per core
P = 128  # SBUF partitions (samples per group)
NGRP = BS // P
K = 8  # sequence positions per chunk
EPS = 1e-5
FP = mybir.dt.float32
BF = mybir.dt.bfloat16
INV_D = 1.0 / D
PARAM_NAMES = ("g1", "b1", "g2", "b2", "g3", "b3")
PARAM_DTYPES = {
    "g1": BF, "b1": BF, "g2": BF, "b2": BF, "g3": BF, "b3": FP,
}

SUB = mybir.AluOpType.subtract
MUL = mybir.AluOpType.mult
ADD = mybir.AluOpType.add
COPY = mybir.ActivationFunctionType.Copy
SQUARE = mybir.ActivationFunctionType.Square
SQRT = mybir.ActivationFunctionType.Sqrt


def _bcast_mid(t, k):
    """[P, D] param tile -> [P, k, D] AP, 0-stride on the middle dim."""
    return bass.AP(tensor=t.tensor, offset=t.offset, ap=[t.ap[0], [0, k], t.ap[1]])


def _bcast_free(t, d):
    """[P, K] stats tile -> [P, K, d] AP, 0-stride on the last dim."""
    return bass.AP(
        tensor=t.tensor, offset=t.offset, ap=[t.ap[0], t.ap[1], [0, d]]
    )


def _build(repeat=1):
    nc = bacc.Bacc("TRN2", target_bir_lowering=False, debug=False, num_devices=NCORES)
    x = nc.declare_dram_parameter("x", [BS, S, D], FP, isOutput=False).ap()
    prm = {
        k: nc.declare_dram_parameter(k, [BS, D], PARAM_DTYPES[k], isOutput=False).ap()
        for k in PARAM_NAMES
    }
    out = nc.declare_dram_parameter("out", [BS, S, D], FP, isOutput=True).ap()

    with TileContext(nc) as tc:
        with contextlib.ExitStack() as stack:
            pp = stack.enter_context(tc.tile_pool(name="params", bufs=2))
            px = stack.enter_context(tc.tile_pool(name="xin", bufs=3))
            po = stack.enter_context(tc.tile_pool(name="yout", bufs=2))
            pi = stack.enter_context(tc.tile_pool(name="inter", bufs=2))
            pzu = stack.enter_context(tc.tile_pool(name="zu", bufs=3))
            pdmp = stack.enter_context(tc.tile_pool(name="dumps", bufs=2))
            ps = stack.enter_context(tc.tile_pool(name="small", bufs=12))
            pc = stack.enter_context(tc.tile_pool(name="singles", bufs=1))
            eps_tile = pc.tile([P, 1], FP)
            nc.vector.memset(eps_tile, EPS * D * D)

            def stats_finish(s, q, tag):
                """[P, K] raw sums -> (m*r, r) for centering z = y*r - m*r.
                Works in un-normalized sums: V = D*q - s^2 = D^2*var,
                r'' = 1/sqrt(V + eps*D^2) = r/D, m*r = s*r'', r = D*r''."""
                a = ps.tile([P, K], FP, tag=f"msq{tag}")
                nc.vector.tensor_tensor(out=a, in0=s, in1=s, op=MUL)
                V = ps.tile([P, K], FP, tag=f"var{tag}")
                nc.vector.scalar_tensor_tensor(
                    out=V, in0=q, scalar=float(D), in1=a, op0=MUL, op1=SUB
                )
                std = ps.tile([P, K], FP, tag=f"std{tag}")
                nc.scalar.activation(out=std, in_=V, func=SQRT, bias=eps_tile)
                rp = ps.tile([P, K], FP, tag=f"rp{tag}")
                nc.vector.reciprocal(out=rp, in_=std)
                mr = ps.tile([P, K], FP, tag=f"mr{tag}")
                nc.vector.tensor_tensor(out=mr, in0=s, in1=rp, op=MUL)
                r = ps.tile([P, K], FP, tag=f"r{tag}")
                nc.vector.tensor_scalar_mul(out=r, in0=rp, scalar1=float(D))
                return mr, r

            def center(src, mr, r, src_tag_dt=BF):
                """z = src*r - m*r via two K-fused broadcast multiplies."""
                t = pzu.tile([P, K, D], BF, tag="z")
                nc.vector.tensor_tensor(
                    out=t, in0=src, in1=_bcast_free(r, D), op=MUL
                )
                z = pzu.tile([P, K, D], BF, tag="z")
                nc.vector.tensor_tensor(
                    out=z, in0=t, in1=_bcast_free(mr, D), op=SUB
                )
                return z

            def ln_stats(y, tag):
                """Sum(y) and Sum(y^2) via K-fused reduce + square-reduce."""
                s = ps.tile([P, K], FP, tag=f"s{tag}")
                nc.vector.tensor_reduce(
                    out=s, in_=y, axis=mybir.AxisListType.X, op=ADD
                )
                sq = pdmp.tile([P, K, D], BF, tag="sq")
                nc.vector.tensor_tensor(out=sq, in0=y, in1=y, op=MUL)
                q = ps.tile([P, K], FP, tag=f"q{tag}")
                nc.vector.tensor_reduce(
                    out=q, in_=sq, axis=mybir.AxisListType.X, op=ADD
                )
                return s, q

            def s0_load(st):
                b0, s0 = st["b0"], st["s0"]
                xt = px.tile([P, K, D], FP)
                nc.sync.dma_start(out=xt, in_=x[b0 : b0 + P, s0 : s0 + K, :])
                st["xt"] = xt

            def s1_xstats(st):
                xt = st["xt"]
                st["sx"], st["qx"] = ln_stats(xt, "x")

            def s2_ln1(st):
                mr1, r1 = stats_finish(st["sx"], st["qx"], "1")
                z = center(st["xt"], mr1, r1)
                u = pzu.tile([P, K, D], BF, tag="u")
                nc.vector.tensor_tensor(
                    out=u, in0=z, in1=_bcast_mid(st["pt"]["g1"], K), op=MUL
                )
                y1 = pi.tile([P, K, D], BF, tag="y1")
                nc.vector.tensor_tensor(
                    out=y1, in0=u, in1=_bcast_mid(st["pt"]["b1"], K), op=ADD
                )
                st["s1"], st["q1"] = ln_stats(y1, "1")
                st["y1"] = y1

            def s3_ln2(st):
                y1 = st["y1"]
                mr2, r2 = stats_finish(st["s1"], st["q1"], "2")
                z2 = center(y1, mr2, r2)
                u2 = pzu.tile([P, K, D], BF, tag="u")
                nc.vector.tensor_tensor(
                    out=u2, in0=z2, in1=_bcast_mid(st["pt"]["g2"], K), op=MUL
                )
                y2 = pi.tile([P, K, D], BF, tag="y2")
                nc.vector.tensor_tensor(
                    out=y2, in0=u2, in1=_bcast_mid(st["pt"]["b2"], K), op=ADD
                )
                st["s2"], st["q2"] = ln_stats(y2, "2")
                st["y2"] = y2

            def s4_ln3(st):
                b0, s0 = st["b0"], st["s0"]
                y2 = st["y2"]
                mr3, r3 = stats_finish(st["s2"], st["q2"], "3")
                z3 = center(y2, mr3, r3)
                u3 = pzu.tile([P, K, D], BF, tag="u")
                nc.vector.tensor_tensor(
                    out=u3, in0=z3, in1=_bcast_mid(st["pt"]["g3"], K), op=MUL
                )
                ot = po.tile([P, K, D], FP)
                nc.gpsimd.tensor_tensor(
                    out=ot, in0=u3, in1=_bcast_mid(st["pt"]["b3"], K), op=ADD
                )
                nc.sync.dma_start(out=out[b0 : b0 + P, s0 : s0 + K, :], in_=ot)

            STAGES = [s0_load, s1_xstats, s2_ln1, s3_ln2, s4_ln3]

            def body():
                pts = []
                for grp in range(NGRP):
                    b0 = grp * P
                    pt = {}
                    for kname in PARAM_NAMES:
                        t = pp.tile([P, D], PARAM_DTYPES[kname], tag=kname)
                        nc.sync.dma_start(out=t, in_=prm[kname][b0 : b0 + P, :])
                        pt[kname] = t
                    pts.append(pt)
                chunks = [
                    {"pt": pts[grp], "b0": grp * P, "s0": c * K}
                    for c in range(S // K)
                    for grp in range(NGRP)
                ]
                n = len(chunks)
                depth = len(STAGES)
                for i in range(n + depth - 1):
                    for d in reversed(range(depth)):
                        ci = i - d
                        if 0 <= ci < n:
                            STAGES[d](chunks[ci])
                for st in chunks:
                    st.clear()

            if repeat == 1:
                body()
            else:
                with tc.For_i(0, repeat, 1):
                    body()
    nc.compile()
    return nc



class _Runner:
    """Persistent compiled SPMD executor for the Bass graph.

    Mirrors bass2jax.run_bass_via_pjrt but keeps the jitted callable and the
    device mesh alive so repeated calls don't retrace/recompile.
    """

    def __init__(self, nc):
        import jax
        import concourse.bass2jax as bass2jax
        from jax.experimental.shard_map import shard_map
        from jax.sharding import Mesh, NamedSharding, PartitionSpec

        bass2jax.install_neuronx_cc_hook()
        self._jax = jax
        self._nc = nc

        partition_name = (
            nc.partition_id_tensor.name if nc.partition_id_tensor else None
        )
        in_names = []
        out_names = []
        out_avals = []
        for alloc in nc.m.functions[0].allocations:
            if not isinstance(alloc, mybir.MemoryLocationSet):
                continue
            name = alloc.memorylocations[0].name
            if alloc.kind == "ExternalInput":
                if name != partition_name:
                    in_names.append(name)
            elif alloc.kind == "ExternalOutput":
                out_names.append(name)
                out_avals.append(
                    jax.core.ShapedArray(
                        tuple(alloc.tensor_shape), mybir.dt.np(alloc.dtype)
                    )
                )
        self.in_names = list(in_names)
        self.out_names = out_names
        self.out_avals = out_avals
        n_params = len(in_names)
        all_in_names = in_names + out_names
        if partition_name is not None:
            all_in_names = all_in_names + [partition_name]

        def _body(*args):
            operands = list(args)
            if partition_name is not None:
                operands.append(bass2jax.partition_id_tensor())
            outs = bass2jax._bass_exec_p.bind(
                *operands,
                out_avals=tuple(out_avals),
                in_names=tuple(all_in_names),
                out_names=tuple(out_names),
                lowering_input_output_aliases=(),
                sim_require_finite=True,
                sim_require_nnan=True,
                nc=nc,
            )
            return tuple(outs)

        devices = jax.devices()[:NCORES]
        self.mesh = Mesh(np.asarray(devices), ("core",))
        self.sharding = NamedSharding(self.mesh, PartitionSpec("core"))
        n_outs = len(out_names)
        donate = tuple(range(n_params, n_params + n_outs))
        self._exec = jax.jit(
            shard_map(
                _body,
                mesh=self.mesh,
                in_specs=(PartitionSpec("core"),) * (n_params + n_outs),
                out_specs=(PartitionSpec("core"),) * n_outs,
                check_rep=False,
            ),
            donate_argnums=donate,
            keep_unused=True,
        )

        def _mk_zeros():
            import jax.numpy as jnp

            return tuple(
                jnp.zeros((NCORES * a.shape[0], *a.shape[1:]), a.dtype)
                for a in out_avals
            )

        self._zeros = jax.jit(
            _mk_zeros, out_shardings=(self.sharding,) * n_outs
        )

    def put_inputs(self, concat_ins):
        """Transfer concatenated (axis0 = NCORES*shard) inputs to devices."""
        return [
            self._jax.device_put(v, self.sharding) for v in concat_ins
        ]

    def run(self, dev_ins):
        """One execution; returns tuple of global output arrays (device)."""
        zeros = self._zeros()
        return self._exec(*dev_ins, *zeros)


_RUNNERS = {}


def get_runner(repeat=1):
    if repeat not in _RUNNERS:
        _RUNNERS[repeat] = _Runner(_build(repeat=repeat))
    return _RUNNERS[repeat]


def host_inputs(
    x,
    pathway_ids,
    compartment_ids,
    cell_type_ids,
    pathway_gamma,
    pathway_beta,
    compartment_gamma,
    compartment_beta,
    cell_type_gamma,
    cell_type_beta,
):
    """Gather per-sample affine rows and cast to the device dtypes."""
    pid = np.asarray(pathway_ids).astype(np.int64)
    cid = np.asarray(compartment_ids).astype(np.int64)
    tid = np.asarray(cell_type_ids).astype(np.int64)
    full = {
        "x": np.ascontiguousarray(np.asarray(x, dtype=np.float32)),
        "g1": np.asarray(pathway_gamma, np.float32)[pid],
        "b1": np.asarray(pathway_beta, np.float32)[pid],
        "g2": np.asarray(compartment_gamma, np.float32)[cid],
        "b2": np.asarray(compartment_beta, np.float32)[cid],
        "g3": np.asarray(cell_type_gamma, np.float32)[tid],
        "b3": np.asarray(cell_type_beta, np.float32)[tid],
    }
    for k in PARAM_NAMES:
        tgt = PARAM_DTYPES[k]
        if tgt == BF:
            full[k] = np.ascontiguousarray(full[k].astype(ml_dtypes.bfloat16))
        else:
            full[k] = np.ascontiguousarray(full[k])
    return full


def kernel(
    x,
    pathway_ids,
    compartment_ids,
    cell_type_ids,
    pathway_gamma,
    pathway_beta,
    compartment_gamma,
    compartment_beta,
    cell_type_gamma,
    cell_type_beta,
    W=None,
    b=None,
    **_unused,
):
    full = host_inputs(
        x,
        pathway_ids,
        compartment_ids,
        cell_type_ids,
        pathway_gamma,
        pathway_beta,
        compartment_gamma,
        compartment_beta,
        cell_type_gamma,
        cell_type_beta,
    )
    runner = get_runner()
    concat_ins = [full[name] for name in runner.in_names]
    dev_ins = runner.put_inputs(concat_ins)
    outs = runner.run(dev_ins)
    return np.asarray(outs[0])



# revision 8
# speedup vs baseline: 1.4283x; 1.4283x over previous
"""Trainium2 Bass kernel for nn_BiologicalNormalization.

Math: three chained per-sample LayerNorms (affine params gathered per-sample
by id on the host). The trailing gated blend ``x*sigmoid(xW+b) +
x*(1-sigmoid(xW+b))`` is mathematically the identity, so the kernel returns
the triple-LayerNorm result directly.

Distribution: pure data parallelism - batch 2048 is split into 8 shards of
256 samples, one per NeuronCore. Per-id affine tables are gathered to
per-sample rows on the host (tiny), so each core only sees dense tensors.

Per-core schedule: position-sliced tiles [128 samples, 512] so per-position
LayerNorm statistics are per-PARTITION scalars. That unlocks the fast DVE
paths: tensor_scalar with two [P,1] scalar APs runs in 4x mode (0.26
ns/elem) and its accum_out rides along for free, so centering is one 4x op
and each plain sum (Sum x, Sum u1, Sum u2) is one 4x copy-with-accumulate.
Sum(y^2) reductions run on the Activation engine (Square+accum), the
gamma-multiplies and beta-adds are 2x tensor_tensor ops on DVE/Pool, and
rsqrt finalization is batched [128,K] on Act. Work is split across
DVE/Act/Pool so no engine exceeds ~2.6us per tile. Sum(y1) is recovered as
Sum(u1) + Sum(beta1) with the beta sums precomputed on host. All I/O is
bf16 (host casts/upcasts); stats and accumulators are fp32.
"""

import contextlib

import ml_dtypes
import numpy as np

import concourse.bass as bass
import concourse.bacc as bacc
import concourse.mybir as mybir
from concourse.tile import TileContext

NCORES = 8
B, S, D = 2048, 128, 512
BS = B // NCORES  # samples per core
P = 128  # SBUF partitions (samples per group)
NGRP = BS // P
K = 8  # sequence positions per chunk
CH = S // K  # chunks per group
EPS = 1e-5
FP = mybir.dt.float32
BF = mybir.dt.bfloat16
INV_D = 1.0 / D
PARAM_NAMES = ("g1", "b1", "g2", "b2", "g3", "b3")

SUB = mybir.AluOpType.subtract
MUL = mybir.AluOpType.mult
ADD = mybir.AluOpType.add
SQUARE = mybir.ActivationFunctionType.Square
SQRT = mybir.ActivationFunctionType.Sqrt


def _build(repeat=1):
    nc = bacc.Bacc("TRN2", target_bir_lowering=False, debug=False, num_devices=NCORES)
    x = nc.declare_dram_parameter("x", [BS, S, D], BF, isOutput=False).ap()
    prm = {
        k: nc.declare_dram_parameter(k, [BS, D], BF, isOutput=False).ap()
        for k in PARAM_NAMES
    }
    sb1 = nc.declare_dram_parameter("sb1", [BS, 1], FP, isOutput=False).ap()
    sb2 = nc.declare_dram_parameter("sb2", [BS, 1], FP, isOutput=False).ap()
    out = nc.declare_dram_parameter("out", [BS, S, D], BF, isOutput=True).ap()

    with TileContext(nc) as tc:
        with contextlib.ExitStack() as stack:
            pp = stack.enter_context(tc.tile_pool(name="params", bufs=2))
            px = stack.enter_context(tc.tile_pool(name="xin", bufs=3))
            po = stack.enter_context(tc.tile_pool(name="yout", bufs=2))
            py = stack.enter_context(tc.tile_pool(name="ychunks", bufs=2))
            pt = stack.enter_context(tc.tile_pool(name="trans", bufs=4))
            pd = stack.enter_context(tc.tile_pool(name="dumps", bufs=3))
            ps = stack.enter_context(tc.tile_pool(name="stats", bufs=3))

            pc = stack.enter_context(tc.tile_pool(name="consts", bufs=1))
            eps_tile = pc.tile([P, 1], FP)
            nc.vector.memset(eps_tile, EPS)

            def body():
                # --- params (both groups resident; tiny DMAs) ---
                pts = []
                for grp in range(NGRP):
                    b0 = grp * P
                    ptile = {}
                    for kname in PARAM_NAMES:
                        t = pp.tile([P, D], BF, tag=kname)
                        nc.sync.dma_start(out=t, in_=prm[kname][b0 : b0 + P, :])
                        ptile[kname] = t
                    for nm, src in (("sb1", sb1), ("sb2", sb2)):
                        t = pp.tile([P, 1], FP, tag=nm)
                        nc.sync.dma_start(out=t, in_=src[b0 : b0 + P, :])
                        ptile[nm] = t
                    pts.append(ptile)

                def s0_load(st):
                    b0, s0 = st["b0"], st["s0"]
                    xt = px.tile([P, K, D], BF)
                    nc.sync.dma_start(out=xt, in_=x[b0 : b0 + P, s0 : s0 + K, :])
                    st["xt"] = xt

                def finish_stats(Su, Q, sb_col, tag):
                    """-> (mu, rhat), each [P,K] fp32."""
                    if sb_col is not None:
                        Stot = ps.tile([P, K], FP, tag=f"St{tag}")
                        nc.vector.tensor_scalar(
                            out=Stot, in0=Su, scalar1=sb_col, scalar2=None, op0=ADD
                        )
                    else:
                        Stot = Su
                    S2 = ps.tile([P, K], FP, tag=f"S2{tag}")
                    nc.vector.tensor_tensor(out=S2, in0=Stot, in1=Stot, op=MUL)
                    V = ps.tile([P, K], FP, tag=f"V{tag}")
                    nc.vector.scalar_tensor_tensor(
                        out=V, in0=Q, scalar=float(D), in1=S2, op0=MUL, op1=SUB
                    )
                    std = ps.tile([P, K], FP, tag=f"sd{tag}")
                    nc.scalar.activation(
                        out=std, in_=V, func=SQRT, bias=eps_tile,
                        scale=float(1.0 / (D * D)),
                    )
                    rhat = ps.tile([P, K], FP, tag=f"r{tag}")
                    nc.vector.reciprocal(out=rhat, in_=std)
                    mu = ps.tile([P, K], FP, tag=f"m{tag}")
                    nc.vector.tensor_scalar(
                        out=mu, in0=Stot, scalar1=INV_D, scalar2=None, op0=MUL
                    )
                    return mu, rhat

                def s1_xstats(st):
                    xt = st["xt"]
                    Sx = ps.tile([P, K], FP, tag="Sx")
                    Qx = ps.tile([P, K], FP, tag="Qx")
                    for j in range(K):
                        xj = xt[:, j, :]
                        dmp = pd.tile([P, D], BF, tag="dSx")
                        nc.vector.tensor_scalar(
                            out=dmp, in0=xj, scalar1=1.0, scalar2=0.0, op0=MUL,
                            op1=ADD, accum_out=Sx[:, j : j + 1],
                        )
                        sqd = pd.tile([P, D], BF, tag="dQx")
                        nc.scalar.activation(
                            out=sqd, in_=xj, func=SQUARE,
                            accum_out=Qx[:, j : j + 1],
                        )
                    st["mux"], st["rx"] = finish_stats(Sx, Qx, None, "x")

                def s2_ln1(st):
                    xt, ptile = st["xt"], st["pt"]
                    mux, rx = st["mux"], st["rx"]
                    y1 = py.tile([P, K, D], BF, tag="y1")
                    Su1 = ps.tile([P, K], FP, tag="Su1")
                    Q1 = ps.tile([P, K], FP, tag="Q1")
                    for j in range(K):
                        t1 = pt.tile([P, D], BF, tag="t1")
                        nc.vector.tensor_scalar(
                            out=t1, in0=xt[:, j, :],
                            scalar1=mux[:, j : j + 1], scalar2=rx[:, j : j + 1],
                            op0=SUB, op1=MUL,
                        )
                        u1 = pt.tile([P, D], BF, tag="u1")
                        nc.vector.tensor_tensor(out=u1, in0=t1, in1=ptile["g1"], op=MUL)
                        du = pd.tile([P, D], BF, tag="dSu1")
                        nc.vector.tensor_scalar(
                            out=du, in0=u1, scalar1=1.0, scalar2=0.0, op0=MUL,
                            op1=ADD, accum_out=Su1[:, j : j + 1],
                        )
                        nc.gpsimd.tensor_tensor(
                            out=y1[:, j, :], in0=u1, in1=ptile["b1"], op=ADD
                        )
                        sqd = pd.tile([P, D], BF, tag="dQ1")
                        nc.scalar.activation(
                            out=sqd, in_=y1[:, j, :], func=SQUARE,
                            accum_out=Q1[:, j : j + 1],
                        )
                    st["mu1"], st["r1"] = finish_stats(Su1, Q1, ptile["sb1"], "1")
                    st["y1"] = y1

                def s3_ln2(st):
                    ptile = st["pt"]
                    y1 = st["y1"]
                    mu1, r1 = st["mu1"], st["r1"]
                    y2 = py.tile([P, K, D], BF, tag="y2")
                    Su2 = ps.tile([P, K], FP, tag="Su2")
                    Q2 = ps.tile([P, K], FP, tag="Q2")
                    for j in range(K):
                        t2 = pt.tile([P, D], BF, tag="t2")
                        nc.vector.tensor_scalar(
                            out=t2, in0=y1[:, j, :],
                            scalar1=mu1[:, j : j + 1], scalar2=r1[:, j : j + 1],
                            op0=SUB, op1=MUL,
                        )
                        u2 = pt.tile([P, D], BF, tag="u2")
                        nc.vector.tensor_tensor(out=u2, in0=t2, in1=ptile["g2"], op=MUL)
                        du = pd.tile([P, D], BF, tag="dSu2")
                        nc.vector.tensor_scalar(
                            out=du, in0=u2, scalar1=1.0, scalar2=0.0, op0=MUL,
                            op1=ADD, accum_out=Su2[:, j : j + 1],
                        )
                        nc.vector.tensor_tensor(
                            out=y2[:, j, :], in0=u2, in1=ptile["b2"], op=ADD
                        )
                        sqd = pd.tile([P, D], BF, tag="dQ2")
                        nc.scalar.activation(
                            out=sqd, in_=y2[:, j, :], func=SQUARE,
                            accum_out=Q2[:, j : j + 1],
                        )
                    st["mu2"], st["r2"] = finish_stats(Su2, Q2, ptile["sb2"], "2")
                    st["y2"] = y2

                def s4_ln3(st):
                    b0, s0, ptile = st["b0"], st["s0"], st["pt"]
                    y2 = st["y2"]
                    mu2, r2 = st["mu2"], st["r2"]
                    ot = po.tile([P, K, D], BF)
                    for j in range(K):
                        t3 = pt.tile([P, D], BF, tag="t3")
                        nc.vector.tensor_scalar(
                            out=t3, in0=y2[:, j, :],
                            scalar1=mu2[:, j : j + 1], scalar2=r2[:, j : j + 1],
                            op0=SUB, op1=MUL,
                        )
                        u3 = pt.tile([P, D], BF, tag="u3")
                        nc.vector.tensor_tensor(out=u3, in0=t3, in1=ptile["g3"], op=MUL)
                        nc.gpsimd.tensor_tensor(
                            out=ot[:, j, :], in0=u3, in1=ptile["b3"], op=ADD
                        )
                    nc.sync.dma_start(out=out[b0 : b0 + P, s0 : s0 + K, :], in_=ot)

                STAGES = [s0_load, s1_xstats, s2_ln1, s3_ln2, s4_ln3]
                chunks = [
                    {"pt": pts[grp], "b0": grp * P, "s0": c * K}
                    for c in range(CH)
                    for grp in range(NGRP)
                ]
                n = len(chunks)
                depth = len(STAGES)
                for i in range(n + depth - 1):
                    for d in reversed(range(depth)):
                        ci = i - d
                        if 0 <= ci < n:
                            STAGES[d](chunks[ci])
                for st in chunks:
                    st.clear()

            if repeat == 1:
                body()
            else:
                with tc.For_i(0, repeat, 1):
                    body()
    nc.compile()
    return nc


class _Runner:
    """Persistent compiled SPMD executor for the Bass graph."""

    def __init__(self, nc):
        import jax
        import concourse.bass2jax as bass2jax
        from jax.experimental.shard_map import shard_map
        from jax.sharding import Mesh, NamedSharding, PartitionSpec

        bass2jax.install_neuronx_cc_hook()
        self._jax = jax
        self._nc = nc

        partition_name = (
            nc.partition_id_tensor.name if nc.partition_id_tensor else None
        )
        in_names = []
        out_names = []
        out_avals = []
        for alloc in nc.m.functions[0].allocations:
            if not isinstance(alloc, mybir.MemoryLocationSet):
                continue
            name = alloc.memorylocations[0].name
            if alloc.kind == "ExternalInput":
                if name != partition_name:
                    in_names.append(name)
            elif alloc.kind == "ExternalOutput":
                out_names.append(name)
                out_avals.append(
                    jax.core.ShapedArray(
                        tuple(alloc.tensor_shape), mybir.dt.np(alloc.dtype)
                    )
                )
        self.in_names = list(in_names)
        self.out_names = out_names
        self.out_avals = out_avals
        n_params = len(in_names)
        all_in_names = in_names + out_names
        if partition_name is not None:
            all_in_names = all_in_names + [partition_name]

        def _body(*args):
            operands = list(args)
            if partition_name is not None:
                operands.append(bass2jax.partition_id_tensor())
            outs = bass2jax._bass_exec_p.bind(
                *operands,
                out_avals=tuple(out_avals),
                in_names=tuple(all_in_names),
                out_names=tuple(out_names),
                lowering_input_output_aliases=(),
                sim_require_finite=True,
                sim_require_nnan=True,
                nc=nc,
            )
            return tuple(outs)

        devices = jax.devices()[:NCORES]
        self.mesh = Mesh(np.asarray(devices), ("core",))
        self.sharding = NamedSharding(self.mesh, PartitionSpec("core"))
        n_outs = len(out_names)
        donate = tuple(range(n_params, n_params + n_outs))
        self._exec = jax.jit(
            shard_map(
                _body,
                mesh=self.mesh,
                in_specs=(PartitionSpec("core"),) * (n_params + n_outs),
                out_specs=(PartitionSpec("core"),) * n_outs,
                check_rep=False,
            ),
            donate_argnums=donate,
            keep_unused=True,
        )

        def _mk_zeros():
            import jax.numpy as jnp

            return tuple(
                jnp.zeros((NCORES * a.shape[0], *a.shape[1:]), a.dtype)
                for a in out_avals
            )

        self._zeros = jax.jit(
            _mk_zeros, out_shardings=(self.sharding,) * n_outs
        )

    def put_inputs(self, concat_ins):
        return [
            self._jax.device_put(v, self.sharding) for v in concat_ins
        ]

    def run(self, dev_ins):
        zeros = self._zeros()
        return self._exec(*dev_ins, *zeros)


_RUNNERS = {}


def get_runner(repeat=1):
    if repeat not in _RUNNERS:
        _RUNNERS[repeat] = _Runner(_build(repeat=repeat))
    return _RUNNERS[repeat]


def host_inputs(
    x,
    pathway_ids,
    compartment_ids,
    cell_type_ids,
    pathway_gamma,
    pathway_beta,
    compartment_gamma,
    compartment_beta,
    cell_type_gamma,
    cell_type_beta,
):
    """Gather per-sample affine rows, cast to device dtypes, precompute
    the per-sample beta sums used to turn Sum(u) into Sum(y)."""
    pid = np.asarray(pathway_ids).astype(np.int64)
    cid = np.asarray(compartment_ids).astype(np.int64)
    tid = np.asarray(cell_type_ids).astype(np.int64)
    b1 = np.asarray(pathway_beta, np.float32)[pid]
    b2 = np.asarray(compartment_beta, np.float32)[cid]
    full = {
        "x": np.asarray(x, dtype=np.float32).astype(ml_dtypes.bfloat16),
        "g1": np.asarray(pathway_gamma, np.float32)[pid].astype(ml_dtypes.bfloat16),
        "b1": b1.astype(ml_dtypes.bfloat16),
        "g2": np.asarray(compartment_gamma, np.float32)[cid].astype(ml_dtypes.bfloat16),
        "b2": b2.astype(ml_dtypes.bfloat16),
        "g3": np.asarray(cell_type_gamma, np.float32)[tid].astype(ml_dtypes.bfloat16),
        "b3": np.asarray(cell_type_beta, np.float32)[tid].astype(ml_dtypes.bfloat16),
        # device adds Sum(beta) to Sum(u); use the bf16-rounded betas so the
        # correction matches what the device actually added elementwise
        "sb1": b1.astype(ml_dtypes.bfloat16).astype(np.float32).sum(
            axis=-1, keepdims=True
        ),
        "sb2": b2.astype(ml_dtypes.bfloat16).astype(np.float32).sum(
            axis=-1, keepdims=True
        ),
    }
    for k in list(full):
        full[k] = np.ascontiguousarray(full[k])
    return full


def kernel(
    x,
    pathway_ids,
    compartment_ids,
    cell_type_ids,
    pathway_gamma,
    pathway_beta,
    compartment_gamma,
    compartment_beta,
    cell_type_gamma,
    cell_type_beta,
    W=None,
    b=None,
    **_unused,
):
    full = host_inputs(
        x,
        pathway_ids,
        compartment_ids,
        cell_type_ids,
        pathway_gamma,
        pathway_beta,
        compartment_gamma,
        compartment_beta,
        cell_type_gamma,
        cell_type_beta,
    )
    runner = get_runner()
    concat_ins = [full[name] for name in runner.in_names]
    dev_ins = runner.put_inputs(concat_ins)
    outs = runner.run(dev_ins)
    return np.asarray(outs[0]).astype(np.float32)


# revision 11
# speedup vs baseline: 1.5886x; 1.1122x over previous
"""Trainium2 Bass kernel for nn_BiologicalNormalization.

Math: three chained per-sample LayerNorms (affine params gathered per-sample
by id on the host). The trailing gated blend ``x*sigmoid(xW+b) +
x*(1-sigmoid(xW+b))`` is mathematically the identity, so the kernel returns
the triple-LayerNorm result directly.

Distribution: pure data parallelism - batch 2048 is split into 8 shards of
256 samples, one per NeuronCore. Per-id affine tables are gathered to
per-sample rows on the host (tiny), so each core only sees dense tensors.

Per-core schedule: position-sliced tiles [128 samples, 512] so per-position
LayerNorm statistics are per-PARTITION scalars. That unlocks the fast DVE
paths: tensor_scalar with two [P,1] scalar APs runs in 4x mode (0.26
ns/elem) and its accum_out rides along for free, so centering is one 4x op
and each plain sum (Sum x, Sum u1, Sum u2) is one 4x copy-with-accumulate.
Sum(y^2) reductions run on the Activation engine (Square+accum), the
gamma-multiplies and beta-adds are 2x tensor_tensor ops on DVE/Pool, and
rsqrt finalization is batched [128,K] on Act. Work is split across
DVE/Act/Pool so no engine exceeds ~2.6us per tile. Sum(y1) is recovered as
Sum(u1) + Sum(beta1) with the beta sums precomputed on host. All I/O is
bf16 (host casts/upcasts); stats and accumulators are fp32.
"""

import contextlib

import ml_dtypes
import numpy as np

import concourse.bass as bass
import concourse.bacc as bacc
import concourse.mybir as mybir
from concourse.tile import TileContext

NCORES = 8
B, S, D = 2048, 128, 512
BS = B // NCORES  # samples per core
P = 128  # SBUF partitions (samples per group)
NGRP = BS // P
K = 8  # sequence positions per chunk
CH = S // K  # chunks per group
EPS = 1e-5
FP = mybir.dt.float32
BF = mybir.dt.bfloat16
INV_D = 1.0 / D
PARAM_NAMES = ("g1", "b1", "g2", "b2", "g3", "b3")

SUB = mybir.AluOpType.subtract
MUL = mybir.AluOpType.mult
ADD = mybir.AluOpType.add
SQUARE = mybir.ActivationFunctionType.Square
SQRT = mybir.ActivationFunctionType.Sqrt


def _build(repeat=1):
    nc = bacc.Bacc("TRN2", target_bir_lowering=False, debug=False, num_devices=NCORES)
    x = nc.declare_dram_parameter("x", [BS, S, D], BF, isOutput=False).ap()
    prm = {
        k: nc.declare_dram_parameter(k, [BS, D], BF, isOutput=False).ap()
        for k in PARAM_NAMES
    }
    sb1 = nc.declare_dram_parameter("sb1", [BS, 1], FP, isOutput=False).ap()
    sb2 = nc.declare_dram_parameter("sb2", [BS, 1], FP, isOutput=False).ap()
    out = nc.declare_dram_parameter("out", [BS, S, D], BF, isOutput=True).ap()

    with TileContext(nc) as tc:
        with contextlib.ExitStack() as stack:
            pp = stack.enter_context(tc.tile_pool(name="params", bufs=2))
            px = stack.enter_context(tc.tile_pool(name="xin", bufs=3))
            pxc = stack.enter_context(tc.tile_pool(name="xc", bufs=2))
            py1 = stack.enter_context(tc.tile_pool(name="y1", bufs=2))
            py1c = stack.enter_context(tc.tile_pool(name="y1c", bufs=2))
            py2 = stack.enter_context(tc.tile_pool(name="y2", bufs=2))
            py2c = stack.enter_context(tc.tile_pool(name="y2c", bufs=2))
            pt = stack.enter_context(tc.tile_pool(name="trans", bufs=4))
            pd = stack.enter_context(tc.tile_pool(name="dumps", bufs=3))
            ps = stack.enter_context(tc.tile_pool(name="stats", bufs=3))
            po = stack.enter_context(tc.tile_pool(name="yout", bufs=2))

            pc = stack.enter_context(tc.tile_pool(name="consts", bufs=1))
            eps_tile = pc.tile([P, 1], FP)
            nc.vector.memset(eps_tile, EPS)

            def _bcast(col, k):
                """[P,1] fp32 -> [P,k] 0-stride broadcast AP."""
                return bass.AP(
                    tensor=col.tensor, offset=col.offset,
                    ap=[col.ap[0], [0, k]],
                )

            def body():
                pts = []
                for grp in range(NGRP):
                    b0 = grp * P
                    ptile = {}
                    for kname in PARAM_NAMES:
                        t = pp.tile([P, D], BF, tag=kname)
                        nc.sync.dma_start(out=t, in_=prm[kname][b0 : b0 + P, :])
                        ptile[kname] = t
                    for nm, srcp in (("sb1", sb1), ("sb2", sb2)):
                        t = pp.tile([P, 1], FP, tag=nm)
                        nc.sync.dma_start(out=t, in_=srcp[b0 : b0 + P, :])
                        ptile[nm] = t
                    pts.append(ptile)

                def s0_load(st):
                    b0, s0 = st["b0"], st["s0"]
                    xt = px.tile([P, K, D], BF)
                    nc.sync.dma_start(out=xt, in_=x[b0 : b0 + P, s0 : s0 + K, :])
                    st["xt"] = xt

                def s1_sx(st):
                    xt = st["xt"]
                    Sx = ps.tile([P, K], FP, tag="Sx")
                    for j in range(K):
                        dmp = pd.tile([P, D], BF, tag="dSx")
                        nc.vector.tensor_scalar(
                            out=dmp, in0=xt[:, j, :], scalar1=1.0, scalar2=0.0,
                            op0=MUL, op1=ADD, accum_out=Sx[:, j : j + 1],
                        )
                    mux = ps.tile([P, K], FP, tag="mux")
                    nc.vector.tensor_scalar(
                        out=mux, in0=Sx, scalar1=INV_D, scalar2=None, op0=MUL
                    )
                    st["mux"] = mux

                def _center_stats(st, src_key, mu_key, dst_pool, dst_key, w_tag):
                    """DVE centers each slice; Act accumulates squares and
                    takes sqrt -> std (all Act-local)."""
                    srct, mu = st[src_key], st[mu_key]
                    ct = dst_pool.tile([P, K, D], BF, tag=dst_key)
                    W = ps.tile([P, K], FP, tag=f"W{w_tag}")
                    for j in range(K):
                        nc.vector.tensor_scalar(
                            out=ct[:, j, :], in0=srct[:, j, :],
                            scalar1=mu[:, j : j + 1], scalar2=1.0,
                            op0=SUB, op1=MUL,
                        )
                        sqd = pd.tile([P, D], BF, tag=f"dQ{w_tag}")
                        nc.scalar.activation(
                            out=sqd, in_=ct[:, j, :], func=SQUARE,
                            accum_out=W[:, j : j + 1],
                        )
                    std = ps.tile([P, K], FP, tag=f"sd{w_tag}")
                    nc.scalar.activation(
                        out=std, in_=W, func=SQRT, bias=eps_tile, scale=INV_D
                    )
                    st[dst_key] = ct
                    st[f"std{w_tag}"] = std

                def s2_cx(st):
                    _center_stats(st, "xt", "mux", pxc, "xc", "x")

                def _ln_apply(st, c_key, std_tag, g_name, b_name, sb_name,
                              y_pool, y_key, su_tag, add_engine):
                    """recip (DVE) + per-slice gamma-mult, rstd-scale (with
                    running sum), beta-add on `add_engine`."""
                    ptile = st["pt"]
                    ct = st[c_key]
                    r = ps.tile([P, K], FP, tag=f"r{std_tag}")
                    nc.vector.reciprocal(out=r, in_=st[f"std{std_tag}"])
                    yt = y_pool.tile([P, K, D], BF, tag=y_key)
                    Su = ps.tile([P, K], FP, tag=f"Su{su_tag}")
                    for j in range(K):
                        u = pt.tile([P, D], BF, tag=f"u{su_tag}")
                        nc.vector.tensor_tensor(
                            out=u, in0=ct[:, j, :], in1=ptile[g_name], op=MUL
                        )
                        yp = pt.tile([P, D], BF, tag=f"yp{su_tag}")
                        nc.vector.tensor_scalar(
                            out=yp, in0=u, scalar1=r[:, j : j + 1], scalar2=0.0,
                            op0=MUL, op1=ADD, accum_out=Su[:, j : j + 1],
                        )
                        add_engine.tensor_tensor(
                            out=yt[:, j, :], in0=yp, in1=ptile[b_name], op=ADD
                        )
                    mu = ps.tile([P, K], FP, tag=f"mu{su_tag}")
                    nc.vector.scalar_tensor_tensor(
                        out=mu, in0=Su, scalar=INV_D, in1=_bcast(ptile[sb_name], K),
                        op0=MUL, op1=ADD,
                    )
                    st[y_key] = yt
                    st[f"mu{su_tag}"] = mu

                def s3_ln1(st):
                    _ln_apply(st, "xc", "x", "g1", "b1", "sb1", py1, "y1", "1",
                              nc.gpsimd)

                def s4_cy1(st):
                    _center_stats(st, "y1", "mu1", py1c, "y1c", "1")

                def s5_ln2(st):
                    _ln_apply(st, "y1c", "1", "g2", "b2", "sb2", py2, "y2", "2",
                              nc.vector)

                def s6_cy2(st):
                    _center_stats(st, "y2", "mu2", py2c, "y2c", "2")

                def s7_ln3(st):
                    b0, s0, ptile = st["b0"], st["s0"], st["pt"]
                    ct = st["y2c"]
                    r = ps.tile([P, K], FP, tag="r2f")
                    nc.vector.reciprocal(out=r, in_=st["std2"])
                    ot = po.tile([P, K, D], BF)
                    for j in range(K):
                        u = pt.tile([P, D], BF, tag="u3")
                        nc.vector.tensor_tensor(
                            out=u, in0=ct[:, j, :], in1=ptile["g3"], op=MUL
                        )
                        yp = pt.tile([P, D], BF, tag="yp3")
                        nc.vector.tensor_scalar(
                            out=yp, in0=u, scalar1=r[:, j : j + 1], scalar2=0.0,
                            op0=MUL, op1=ADD,
                        )
                        nc.gpsimd.tensor_tensor(
                            out=ot[:, j, :], in0=yp, in1=ptile["b3"], op=ADD
                        )
                    nc.sync.dma_start(out=out[b0 : b0 + P, s0 : s0 + K, :], in_=ot)

                STAGES = [s0_load, s1_sx, s2_cx, s3_ln1, s4_cy1, s5_ln2,
                          s6_cy2, s7_ln3]
                chunks = [
                    {"pt": pts[grp], "b0": grp * P, "s0": c * K}
                    for c in range(CH)
                    for grp in range(NGRP)
                ]
                n = len(chunks)
                depth = len(STAGES)
                for i in range(n + depth - 1):
                    for d in reversed(range(depth)):
                        ci = i - d
                        if 0 <= ci < n:
                            STAGES[d](chunks[ci])
                for st in chunks:
                    st.clear()

            if repeat == 1:
                body()
            else:
                with tc.For_i(0, repeat, 1):
                    body()
    nc.compile()
    return nc


class _Runner:
    """Persistent compiled SPMD executor for the Bass graph."""

    def __init__(self, nc):
        import jax
        import concourse.bass2jax as bass2jax
        from jax.experimental.shard_map import shard_map
        from jax.sharding import Mesh, NamedSharding, PartitionSpec

        bass2jax.install_neuronx_cc_hook()
        self._jax = jax
        self._nc = nc

        partition_name = (
            nc.partition_id_tensor.name if nc.partition_id_tensor else None
        )
        in_names = []
        out_names = []
        out_avals = []
        for alloc in nc.m.functions[0].allocations:
            if not isinstance(alloc, mybir.MemoryLocationSet):
                continue
            name = alloc.memorylocations[0].name
            if alloc.kind == "ExternalInput":
                if name != partition_name:
                    in_names.append(name)
            elif alloc.kind == "ExternalOutput":
                out_names.append(name)
                out_avals.append(
                    jax.core.ShapedArray(
                        tuple(alloc.tensor_shape), mybir.dt.np(alloc.dtype)
                    )
                )
        self.in_names = list(in_names)
        self.out_names = out_names
        self.out_avals = out_avals
        n_params = len(in_names)
        all_in_names = in_names + out_names
        if partition_name is not None:
            all_in_names = all_in_names + [partition_name]

        def _body(*args):
            operands = list(args)
            if partition_name is not None:
                operands.append(bass2jax.partition_id_tensor())
            outs = bass2jax._bass_exec_p.bind(
                *operands,
                out_avals=tuple(out_avals),
                in_names=tuple(all_in_names),
                out_names=tuple(out_names),
                lowering_input_output_aliases=(),
                sim_require_finite=True,
                sim_require_nnan=True,
                nc=nc,
            )
            return tuple(outs)

        devices = jax.devices()[:NCORES]
        self.mesh = Mesh(np.asarray(devices), ("core",))
        self.sharding = NamedSharding(self.mesh, PartitionSpec("core"))
        n_outs = len(out_names)
        donate = tuple(range(n_params, n_params + n_outs))
        self._exec = jax.jit(
            shard_map(
                _body,
                mesh=self.mesh,
                in_specs=(PartitionSpec("core"),) * (n_params + n_outs),
                out_specs=(PartitionSpec("core"),) * n_outs,
                check_rep=False,
            ),
            donate_argnums=donate,
            keep_unused=True,
        )

        def _mk_zeros():
            import jax.numpy as jnp

            return tuple(
                jnp.zeros((NCORES * a.shape[0], *a.shape[1:]), a.dtype)
                for a in out_avals
            )

        self._zeros = jax.jit(
            _mk_zeros, out_shardings=(self.sharding,) * n_outs
        )

    def put_inputs(self, concat_ins):
        return [
            self._jax.device_put(v, self.sharding) for v in concat_ins
        ]

    def run(self, dev_ins):
        zeros = self._zeros()
        return self._exec(*dev_ins, *zeros)


_RUNNERS = {}


def get_runner(repeat=1):
    if repeat not in _RUNNERS:
        _RUNNERS[repeat] = _Runner(_build(repeat=repeat))
    return _RUNNERS[repeat]


def host_inputs(
    x,
    pathway_ids,
    compartment_ids,
    cell_type_ids,
    pathway_gamma,
    pathway_beta,
    compartment_gamma,
    compartment_beta,
    cell_type_gamma,
    cell_type_beta,
):
    """Gather per-sample affine rows, cast to device dtypes, precompute
    the per-sample beta sums used to turn Sum(u) into Sum(y)."""
    pid = np.asarray(pathway_ids).astype(np.int64)
    cid = np.asarray(compartment_ids).astype(np.int64)
    tid = np.asarray(cell_type_ids).astype(np.int64)
    b1 = np.asarray(pathway_beta, np.float32)[pid]
    b2 = np.asarray(compartment_beta, np.float32)[cid]
    full = {
        "x": np.asarray(x, dtype=np.float32).astype(ml_dtypes.bfloat16),
        "g1": np.asarray(pathway_gamma, np.float32)[pid].astype(ml_dtypes.bfloat16),
        "b1": b1.astype(ml_dtypes.bfloat16),
        "g2": np.asarray(compartment_gamma, np.float32)[cid].astype(ml_dtypes.bfloat16),
        "b2": b2.astype(ml_dtypes.bfloat16),
        "g3": np.asarray(cell_type_gamma, np.float32)[tid].astype(ml_dtypes.bfloat16),
        "b3": np.asarray(cell_type_beta, np.float32)[tid].astype(ml_dtypes.bfloat16),
        # device computes mu = Su/D + sb; ship Sum(beta)/D using the
        # bf16-rounded betas the device actually adds elementwise
        "sb1": b1.astype(ml_dtypes.bfloat16).astype(np.float32).sum(
            axis=-1, keepdims=True
        ) / D,
        "sb2": b2.astype(ml_dtypes.bfloat16).astype(np.float32).sum(
            axis=-1, keepdims=True
        ) / D,
    }
    for k in list(full):
        full[k] = np.ascontiguousarray(full[k])
    return full


def kernel(
    x,
    pathway_ids,
    compartment_ids,
    cell_type_ids,
    pathway_gamma,
    pathway_beta,
    compartment_gamma,
    compartment_beta,
    cell_type_gamma,
    cell_type_beta,
    W=None,
    b=None,
    **_unused,
):
    full = host_inputs(
        x,
        pathway_ids,
        compartment_ids,
        cell_type_ids,
        pathway_gamma,
        pathway_beta,
        compartment_gamma,
        compartment_beta,
        cell_type_gamma,
        cell_type_beta,
    )
    runner = get_runner()
    concat_ins = [full[name] for name in runner.in_names]
    dev_ins = runner.put_inputs(concat_ins)
    outs = runner.run(dev_ins)
    return np.asarray(outs[0]).astype(np.float32)
